# revision 40
# baseline (speedup 1.0000x reference)
"""Trainium2 Bass kernel for PVT-style spatial-reduction attention.

Reference computation (B=4, N=4096, C=512, 8 heads, head_dim=64):
  q = (x @ q_w.T) * hd**-0.5                    -> [B, N, C]
  x_ = depthwise_conv2x2_stride2(x as NCHW 64x64) + sr_b -> [B, M=1024, C]
  x_ = layernorm(x_) * ln_g + ln_b
  k, v = split(x_ @ kv_w.T)                      -> [B, nh, M, hd] each
  out = softmax(q k^T) v                         -> [B, N, C]
  out = out @ proj_w.T + proj_b
Sharding: 8 cores, core d handles batch b=d//2, query-half h=d%2 (2048
queries).  Each core computes its output slice independently (the small KV
path is recomputed per half); the host concatenates.  No collectives.

On-device layout is "transposed" (channels on partitions, tokens in the
free dimension) so every matmul contracts over the partition dim.
Attention per head-pair: S^T via row-tiled K=64 concurrent matmuls, exp on
ScalarE, PV per head with the softmax denominator folded in as a 65th
weight column of ones (head A outputs [O_A(64 rows); s_A], head B uses
[1|0*63|v_B] so O_B lands on partitions 64-127 and s_B on partition 0).
The depthwise conv runs on the PE as accumulating diagonal matmuls; the
LayerNorm affine is folded into the K/V weights host-side (softmax is
invariant to the K bias; the V bias folds into proj_b).  One exp tile per
chunk runs on the DVE via the Schraudolph bit-trick to relieve the
saturated ScalarE, with PV matmuls deferred two steps so the PE never
waits on it.  Denominator reciprocals run per chunk: DMA partition-spread
of the two s rows into [32,32], one DVE reciprocal, spread-back +
stride-0-broadcast DMAs, one deferred DVE multiply.
"""

import numpy as np

B, N, C = 4, 4096, 512
NH, HD = 8, 64
M = 1024          # (64/2) * (64/2) spatial-reduced tokens
NHALF = 2048      # queries per core
LN_EPS = 1e-5

NQT = NHALF // 128
KT = C // 128     # 4 c-tiles
MT = M // 128     # 8 m-tiles
NCH = NHALF // 512
PW = 193          # per-pair vhat width: [v_A|1] (65) + [1|0*63|v_B] (128)

_cache = {}


def _build_nc():
    import concourse.tile as tile
    from concourse import bacc, mybir
    from concourse.masks import make_identity

    f32 = mybir.dt.float32
    bf16 = mybir.dt.bfloat16
    AF = mybir.ActivationFunctionType
    OP = mybir.AluOpType

    # Pin Exp/Ln/Square to the one ACT table set that contains all three
    # (natural_log_exp_and_others); otherwise the set chooser alternates
    # between sets and pays a ~2.7us ACT_TABLE_LOAD per switch.
    import concourse.bacc as bacc_mod
    if not hasattr(bacc_mod, "_orig_get_activation_tables"):
        bacc_mod._orig_get_activation_tables = bacc_mod.get_activation_tables

        def _pinned_tables(arch):
            d = bacc_mod._orig_get_activation_tables(arch)
            strip = {AF.Exp, AF.Ln, AF.Square}
            return {
                name: (funcs if name == "natural_log_exp_and_others"
                       else funcs - strip)
                for name, funcs in d.items()
            }

        bacc_mod.get_activation_tables = _pinned_tables

    nc = bacc.Bacc("TRN2", target_bir_lowering=False, debug=False)

    x_d = nc.dram_tensor("xT", [C, N], bf16, kind="ExternalInput")
    qw_d = nc.dram_tensor("q_wT", [C, C], bf16, kind="ExternalInput")
    kw_d = nc.dram_tensor("k_wT", [C, C], bf16, kind="ExternalInput")
    vw_d = nc.dram_tensor("v_wT", [C, C], bf16, kind="ExternalInput")
    pw_d = nc.dram_tensor("p_wT", [C, C], bf16, kind="ExternalInput")
    chan_d = nc.dram_tensor("chan", [C, 8], f32, kind="ExternalInput")
    pb_d = nc.dram_tensor("p_b", [1, C], f32, kind="ExternalInput")
    out_d = nc.dram_tensor("out", [NHALF, C], f32, kind="ExternalOutput")

    with tile.TileContext(nc) as tc:
        _cms = {}

        def pool(name, bufs=1, space="SBUF", side=None):
            cm = tc.tile_pool(name=name, bufs=bufs, space=space, side=side)
            p = cm.__enter__()
            _cms[id(p)] = cm
            return p

        def close(*pools):
            for p in pools:
                _cms.pop(id(p)).__exit__(None, None, None)

        consts = pool("consts")
        ones_f = consts.tile([128, 128], f32, tag="ones_f", name="ones_f")
        nc.vector.memset(ones_f[:], 1.0)
        ones128 = consts.tile([128, 128], bf16, tag="ones128", name="ones128")
        nc.vector.tensor_copy(ones128[:], ones_f[:])
        zero_t = consts.tile([128, 1], f32, tag="zero_t", name="zero_t")
        nc.vector.memset(zero_t[:], 0.0)
        eps_t = consts.tile([128, 1], f32, tag="eps_t", name="eps_t")
        nc.vector.memset(eps_t[:], LN_EPS)
        # chan params per k-tile: cols 0-3 conv taps, 4 sr_b
        chan_t = consts.tile([128, 8 * KT], f32, tag="chan", name="chan_t")
        nc.sync.dma_start(
            chan_t[:].rearrange("p (k c) -> p k c", k=KT),
            chan_d.ap().rearrange("(k p) c -> p k c", p=128))
        chan = [chan_t[:, 8 * k:8 * (k + 1)] for k in range(KT)]
        pb_bc = consts.tile([128, C], f32, tag="pb", name="pb")
        nc.sync.dma_start(pb_bc[:], pb_d.ap().to_broadcast([128, C]))
        ident_b = consts.tile([128, 128], bf16, tag="ident", name="ident_b")
        make_identity(nc, ident_b[:])
        # per-(k, tap) diagonal conv-weight matrices: diag(chan[k][:, tap])
        diag = [[consts.tile([128, 128], bf16, tag=f"dg{k}_{t}",
                             name=f"dg{k}_{t}") for t in range(4)]
                for k in range(KT)]
        for k in range(KT):
            for t in range(4):
                nc.vector.tensor_scalar_mul(diag[k][t][:], ident_b[:],
                                            chan[k][:, t:t + 1])

        # =============== phase T: DMA-transpose loads ======================
        xT_pool = pool("xTp", side="right")
        xT = xT_pool.tile([128, KT * N], bf16, tag="xT", name="xT")
        pw_pool = pool("pwp")
        pw_t = pw_pool.tile([128, KT * C], bf16, tag="pw", name="pw_t")
        # kvs pool holds qT (written in phase Q) plus kTt/vhat (phase K); it
        # stays open through phase A.
        kvs = pool("kvs")
        qT = [kvs.tile([128, NHALF], bf16, tag=f"qT{k}", name=f"qT{k}")
              for k in range(KT)]
        qw_pool = pool("qwp")
        qw_t = qw_pool.tile([128, KT * C], bf16, tag="qw", name="qw_t")
        # load order: first Q-proj inputs (weights block 0 + the query
        # halves of each k-tile), then the rest; proj weights last
        nc.sync.dma_start(qw_t[:, 0:C], qw_d.ap()[0:128, :])
        for k in range(KT):
            nc.sync.dma_start(xT[:, N * k:N * k + NHALF],
                              x_d.ap()[128 * k:128 * (k + 1), 0:NHALF])
        for k in range(1, KT):
            nc.sync.dma_start(qw_t[:, C * k:C * (k + 1)],
                              qw_d.ap()[128 * k:128 * (k + 1), :])
        for k in range(KT):
            nc.sync.dma_start(xT[:, N * k + NHALF:N * (k + 1)],
                              x_d.ap()[128 * k:128 * (k + 1), NHALF:N])
        nc.sync.dma_start(
            pw_t[:].rearrange("p (k j) -> p k j", k=KT),
            pw_d.ap().rearrange("(k p) j -> p k j", p=128))
        qw = [qw_t[:, C * k:C * (k + 1)] for k in range(KT)]
        pw = [pw_t[:, C * k:C * (k + 1)] for k in range(KT)]
        # =============== phase Q: q^T = q_wT.T @ xq^T ======================
        q_psum = pool("q_ps", bufs=2, space="PSUM")
        for co in range(KT):
            for ch in range(NCH):
                ps = q_psum.tile([128, 512], f32, tag="q", name="q")
                for k in range(KT):
                    nc.tensor.matmul(
                        ps[:],
                        qw[k][:, 128 * co:128 * (co + 1)],
                        xT[:, N * k + 512 * ch:N * k + 512 * (ch + 1)],
                        start=(k == 0), stop=(k == KT - 1),
                    )
                nc.vector.tensor_copy(qT[co][:, 512 * ch:512 * (ch + 1)],
                                      ps[:])
        close(q_psum, qw_pool)

        # =============== phase C: depthwise conv + layernorm ===============
        xn_pool = pool("xnp")
        xn = [xn_pool.tile([128, M], bf16, tag=f"xn{k}", name=f"xn{k}")
              for k in range(KT)]

        cv = pool("cv", bufs=2)
        yt_pool = pool("ytp")
        yt = [yt_pool.tile([128, M], bf16, tag=f"yt{k}", name=f"yt{k}")
              for k in range(KT)]
        st_psum = pool("st_ps", space="PSUM")
        SY = st_psum.tile([128, M], f32, tag="SY", name="SY")
        SY2 = st_psum.tile([128, M], f32, tag="SY2", name="SY2")

        cv_psum = pool("cv_ps", bufs=1, space="PSUM")
        for k in range(KT):
            img = xT[:, N * k:N * (k + 1)].rearrange(
                "p (i a j b) -> p a b i j", i=32, a=2, j=32, b=2)
            # depthwise conv as 4 accumulating diagonal matmuls per half
            acc = cv_psum.tile([128, M], f32, tag="cacc", name="cacc")
            for h in range(2):
                isl = slice(16 * h, 16 * (h + 1))
                for tap in range(4):
                    nc.tensor.matmul(
                        acc[:, 512 * h:512 * (h + 1)], diag[k][tap][:],
                        img[:, tap // 2, tap % 2][:, isl, :],
                        start=(tap == 0), stop=(tap == 3))
            # bias add (+ cast to bf16) on ScalarE straight from PSUM
            nc.scalar.activation(yt[k][:], acc[:], AF.Identity,
                                 bias=chan[k][:, 4:5])
            ysq = cv.tile([128, M], bf16, tag="ysq", name="ysq")
            nc.vector.tensor_mul(ysq[:], yt[k][:], yt[k][:])
            for ch2 in range(M // 512):
                s_ = slice(512 * ch2, 512 * (ch2 + 1))
                nc.tensor.matmul(SY[:, s_], ones128[:], yt[k][:, s_],
                                 start=(k == 0), stop=(k == KT - 1))
                nc.tensor.matmul(SY2[:, s_], ones128[:], ysq[:, s_],
                                 start=(k == 0), stop=(k == KT - 1))
        close(cv_psum)

        # r = 1/std = exp(-0.5*ln(SY2/C - (SY/C)^2 + eps)); the division by
        # std is NOT applied to xn -- it factors out of the K/V projections
        # (contraction over c) and is applied to their outputs instead.
        mu2 = cv.tile([128, M], f32, tag="stat", name="mu2")
        nc.scalar.activation(mu2[:], SY[:], AF.Square, bias=zero_t[:],
                             scale=1.0 / C)
        var = cv.tile([128, M], f32, tag="stat", name="var")
        nc.vector.scalar_tensor_tensor(
            var[:], SY2[:], 1.0 / C, mu2[:], op0=OP.mult, op1=OP.subtract)
        lgv = cv.tile([128, M], f32, tag="stat", name="lgv")
        nc.scalar.activation(lgv[:], var[:], AF.Ln, bias=eps_t[:])
        inv_b = cv.tile([128, M], bf16, tag="invb", name="inv_b")
        nc.scalar.activation(inv_b[:], lgv[:], AF.Exp, bias=zero_t[:],
                             scale=-0.5)

        mu_neg = cv.tile([128, M], bf16, tag="mneg", name="mu_neg")
        nc.vector.tensor_scalar_mul(mu_neg[:], SY[:], -1.0 / C)
        for k in range(KT):
            t1 = cv.tile([128, M], bf16, tag="t1", name="t1")
            nc.vector.tensor_add(t1[:], mu_neg[:], yt[k][:])
            nc.vector.tensor_mul(xn[k][:], t1[:], inv_b[:])
        close(st_psum, yt_pool)
        close(xT_pool)

        # =============== phase K: k^T and v-hat projections ================
        kTt = [kvs.tile([128, M], bf16, tag=f"kT{k}", name=f"kT{k}")
               for k in range(KT)]
        # vhat[mt]: [128, 4*PW] bf16, per pair p:
        #   cols p*PW    .. p*PW+63  : v_A           (head 2p)
        #   col  p*PW+64             : ones  -> s_A on out partition 64
        #   col  p*PW+65             : ones  -> s_B on out partition 0
        #   cols p*PW+66 .. p*PW+128 : zeros
        #   cols p*PW+129.. p*PW+192 : v_B           (head 2p+1)
        vhat = [kvs.tile([128, 4 * PW], bf16, tag=f"vh{m}", name=f"vh{m}")
                for m in range(MT)]
        kvw_pool = pool("kvw")
        kw_t = kvw_pool.tile([128, KT * C], bf16, tag="kw", name="kw_t")
        vw_t = kvw_pool.tile([128, KT * C], bf16, tag="vw", name="vw_t")
        nc.sync.dma_start(
            kw_t[:].rearrange("p (k j) -> p k j", k=KT),
            kw_d.ap().rearrange("(k p) j -> p k j", p=128))
        nc.sync.dma_start(
            vw_t[:].rearrange("p (k j) -> p k j", k=KT),
            vw_d.ap().rearrange("(k p) j -> p k j", p=128))
        kw = [kw_t[:, C * k:C * (k + 1)] for k in range(KT)]
        vw = [vw_t[:, C * k:C * (k + 1)] for k in range(KT)]
        kv_psum = pool("kv_ps", bufs=4, space="PSUM")
        for p in range(KT):
            for ch2 in range(M // 512):
                s_ = slice(512 * ch2, 512 * (ch2 + 1))
                kps = kv_psum.tile([128, 512], f32, tag="kv", name="kps")
                for k in range(KT):
                    nc.tensor.matmul(
                        kps[:], kw[k][:, 128 * p:128 * (p + 1)],
                        xn[k][:, s_],
                        start=(k == 0), stop=(k == KT - 1))
                nc.vector.tensor_copy(kTt[p][:, s_], kps[:])
        for mt in range(MT):
            nc.vector.memset(vhat[mt][:], 0.0)
            v4 = vhat[mt][:].rearrange("p (h x) -> p h x", h=4)
            nc.vector.memset(v4[:, :, 64:66], 1.0)
            ps = kv_psum.tile([128, 512], f32, tag="kv", name="kv")
            for k in range(KT):
                nc.tensor.matmul(
                    ps[:], xn[k][:, 128 * mt:128 * (mt + 1)], vw[k][:],
                    start=(k == 0), stop=(k == KT - 1))
            ps4 = ps[:].rearrange("p (pr two d) -> p pr two d", two=2, d=64)
            nc.vector.tensor_copy(v4[:, :, 0:64], ps4[:, :, 0, :])
            nc.vector.tensor_copy(v4[:, :, 129:193], ps4[:, :, 1, :])
        close(kv_psum)

        # =============== phase A: attention + pipelined epilogues ==========
        OT_pool = pool("OTp", side="right")
        OT = [OT_pool.tile([128, NHALF], bf16, tag=f"OT{p}", name=f"OT{p}")
              for p in range(KT)]
        sr_pool = pool("srp", side="right", bufs=2)
        ss_pool = pool("ssp", side="right", bufs=2)
        bc_pool = pool("bcp", bufs=4)

        ppool = pool("ptile", bufs=6)
        S_psum = pool("S_ps", bufs=2, space="PSUM")
        O_psum = pool("O_ps", bufs=2, space="PSUM")

        pend_pv = []  # PV states deferred two steps behind their exp

        def emit_pv(pv):
            """PV pair for one mt, deferred one step so the next QK sits
            ahead of it in the PE stream."""
            p, mt, pt, o_t = pv
            nc.tensor.matmul(
                o_t[0:65, 0:512], vhat[mt][:, PW * p:PW * p + 65],
                pt[:, 0:512],
                start=(mt == 0), stop=(mt == MT - 1))
            nc.tensor.matmul(
                o_t[0:128, 512:1024], vhat[mt][:, PW * p + 65:PW * (p + 1)],
                pt[:, 512:1024],
                start=(mt == 0), stop=(mt == MT - 1))

        def chunk_copies(st):
            """Copy O rows + s rows out of a finished chunk's PSUM.  Deferred
            until the chunk's last (deferred) PV matmul has been emitted."""
            cp, nsl, o_t, srow, _ = st
            nc.vector.tensor_copy(OT[cp][0:64, nsl], o_t[0:64, 0:512])
            nc.vector.tensor_copy(OT[cp][64:128, nsl],
                                  o_t[64:128, 512:1024])
            nc.vector.tensor_copy(srow[64:65, nsl], o_t[64:65, 0:512])
            nc.vector.tensor_copy(srow[0:1, nsl], o_t[0:1, 512:1024])

        def chunk_recip(st):
            """Per-chunk batched 1/s: gather the chunk's two s rows into
            [32,32], one DVE reciprocal, spread-back + broadcast DMAs."""
            cp, nsl, o_t, srow, st_bc = st
            sstack = ss_pool.tile([128, 32], bf16, tag="sst", name="sstack")
            nc.sync.dma_start(
                sstack[0:16, 0:32],
                srow[64:65, nsl].rearrange("p (a b) -> p a b", a=16))
            nc.sync.dma_start(
                sstack[16:32, 0:32],
                srow[0:1, nsl].rearrange("p (a b) -> p a b", a=16))
            sinv_f = ss_pool.tile([128, 32], f32, tag="sif", name="sinv_f")
            nc.vector.reciprocal(sinv_f[0:32, :], sstack[0:32, :])
            sinv_b = ss_pool.tile([128, 32], bf16, tag="sib", name="sinv_b")
            nc.vector.tensor_copy(sinv_b[0:32, :], sinv_f[0:32, :])
            sqs = ss_pool.tile([128, 512], bf16, tag="sqs", name="sqs")
            nc.sync.dma_start(
                sqs[0:1, :].rearrange("p (a b) -> p a b", a=16),
                sinv_b[0:16, :])
            nc.sync.dma_start(
                sqs[64:65, :].rearrange("p (a b) -> p a b", a=16),
                sinv_b[16:32, :])
            bc_t = bc_pool.tile([128, 512], bf16, tag="bc", name="bc")
            nc.sync.dma_start(
                bc_t[0:64, :],
                sqs[0:1, :].unsqueeze(1).broadcast_to([1, 64, 512]))
            nc.sync.dma_start(
                bc_t[64:128, :],
                sqs[64:65, :].unsqueeze(1).broadcast_to([1, 64, 512]))
            st_bc.append(bc_t)

        def chunk_mul(st):
            cp, nsl, o_t, srow, st_bc = st
            nc.vector.tensor_mul(OT[cp][:, nsl], OT[cp][:, nsl],
                                 st_bc[0][:])

        # rolling per-chunk pipeline: chunk t's PSUM->SBUF copies are
        # emitted at chunk t+1 (after its deferred last PV), the 1/s
        # pipeline right after, the normalize multiply at chunk t+2
        todo_copies = []   # chunk states awaiting copies+recip
        todo_mul = []      # chunk states awaiting the normalize multiply

        for p in range(KT):
            # srow: s_A rows live on partition 64 (col 64 of vhat-A), s_B
            # rows on partition 0; 4 chunks of 512 each.
            srow = sr_pool.tile([128, NCH * 512], bf16, tag="srow",
                                name="srow")
            for ch in range(NCH):
                nsl = slice(512 * ch, 512 * (ch + 1))
                o_t = O_psum.tile([128, 1024], f32, tag="O", name="o_t")
                for mt in range(MT):
                    msl = slice(128 * mt, 128 * (mt + 1))
                    S_ps = S_psum.tile([128, 1024], f32, tag="S", name="S_ps")
                    nc.tensor.matmul(
                        S_ps[:, 0:512],
                        kTt[p][0:64, msl], qT[p][0:64, nsl],
                        start=True, stop=True, tile_position=(0, 0))
                    nc.tensor.matmul(
                        S_ps[:, 512:1024],
                        kTt[p][64:128, msl], qT[p][64:128, nsl],
                        start=True, stop=True, tile_position=(64, 0))
                    pt = ppool.tile([128, 1024], bf16, tag="pt", name="pt")
                    if mt == 0:
                        # Schraudolph bit-trick exp on DVE for one tile per
                        # chunk (~3% sawtooth error that largely cancels
                        # between numerator and denominator); offloads the
                        # saturated ScalarE
                        tsch = ppool.tile([128, 1024], mybir.dt.int32,
                                          tag="sch", name="tsch")
                        nc.vector.tensor_scalar(
                            tsch[:], S_ps[:], 12102203.162, 1064866805.0,
                            op0=OP.mult, op1=OP.add)
                        nc.vector.tensor_copy(pt[:], tsch[:].bitcast(f32))
                    else:
                        nc.scalar.activation(pt[:], S_ps[:], AF.Exp,
                                             bias=zero_t[:])
                    if len(pend_pv) >= 2:
                        emit_pv(pend_pv.pop(0))
                    pend_pv.append((p, mt, pt, o_t))
                    if mt == 2 and todo_mul:
                        chunk_mul(todo_mul.pop(0))
                    if mt == 3 and todo_copies:
                        st = todo_copies.pop(0)
                        chunk_copies(st)
                        chunk_recip(st)
                        todo_mul.append(st)
                hist_st = (p, nsl, o_t, srow, [])
                todo_copies.append(hist_st)

        # flush the tail of the pipeline; pending multiplies go first so
        # the projection over their OT slices is not blocked behind the
        # last chunk's reciprocal chain in the DVE queue
        for pv in pend_pv:
            emit_pv(pv)
        last = todo_copies.pop(0)
        chunk_copies(last)
        for st in todo_mul:
            chunk_mul(st)
        todo_mul = []
        chunk_recip(last)
        chunk_mul(last)
        close(O_psum, S_psum, ppool, bc_pool, ss_pool, sr_pool, kvw_pool,
              cv, xn_pool, kvs)

        # =============== phase P: output projection ========================
        opool = pool("outp", bufs=3)
        pj_psum = pool("pj_ps", bufs=4, space="PSUM")
        for nt in range(NQT):
            ps = pj_psum.tile([128, 512], f32, tag="pj", name="pj")
            for p in range(KT):
                nc.tensor.matmul(
                    ps[:], OT[p][:, 128 * nt:128 * (nt + 1)], pw[p][:],
                    start=(p == 0), stop=(p == KT - 1))
            ob = opool.tile([128, 512], f32, tag="ob", name="ob")
            nc.vector.tensor_add(ob[:], ps[:], pb_bc[:])
            nc.sync.dma_start(out_d.ap()[128 * nt:128 * (nt + 1), :], ob[:])
        close(pj_psum, opool, pw_pool, OT_pool, consts)

    nc.compile()
    return nc


def _get_nc():
    if "nc" not in _cache:
        _cache["nc"] = _build_nc()
    return _cache["nc"]


def _make_in_maps(x, q_w, kv_w, proj_w, proj_b, sr_w, sr_b, ln_g, ln_b):
    import ml_dtypes
    bf = ml_dtypes.bfloat16

    x = np.asarray(x, np.float32)
    scale = HD ** -0.5
    ln_g = np.asarray(ln_g, np.float32)
    ln_b = np.asarray(ln_b, np.float32)
    proj_w = np.asarray(proj_w, np.float32)
    q_wT = np.ascontiguousarray((np.asarray(q_w, np.float32).T * scale)
                                .astype(bf))
    kv_w = np.asarray(kv_w, np.float32)
    # LayerNorm affine folded into the weights: the K-side bias drops out
    # (softmax is invariant to a per-query constant), the V-side bias is a
    # constant over tokens and folds into proj_b.
    k_wT = np.ascontiguousarray((kv_w[:C] * ln_g[None, :]).T.astype(bf))
    v_wT = np.ascontiguousarray((kv_w[C:] * ln_g[None, :]).T.astype(bf))
    p_wT = np.ascontiguousarray(proj_w.T.astype(bf))
    chan = np.zeros((C, 8), np.float32)
    sr_w = np.asarray(sr_w, np.float32)
    for di in range(2):
        for dj in range(2):
            chan[:, di * 2 + dj] = sr_w[:, 0, di, dj]
    chan[:, 4] = np.asarray(sr_b, np.float32)
    cv = kv_w[C:] @ ln_b
    p_b = (np.asarray(proj_b, np.float32) + proj_w @ cv).reshape(1, C)

    in_maps = []
    for d in range(8):
        b, half = d // 2, d % 2
        # roll so this core's query half sits at columns [0, NHALF); the
        # induced m-token permutation of the KV path is attention-invariant
        # and the roll (32 image rows) preserves the conv's 2x2 windows
        xbT = np.ascontiguousarray(
            np.roll(x[b].T, -half * NHALF, axis=1).astype(bf))
        in_maps.append({
            "xT": xbT,
            "q_wT": q_wT, "k_wT": k_wT, "v_wT": v_wT, "p_wT": p_wT,
            "chan": chan, "p_b": p_b,
        })
    return in_maps


def kernel(x, q_w, kv_w, proj_w, proj_b, sr_w, sr_b, ln_g, ln_b, H, W):
    from concourse.bass_utils import run_bass_kernel_spmd

    nc = _get_nc()
    in_maps = _make_in_maps(x, q_w, kv_w, proj_w, proj_b, sr_w, sr_b,
                            ln_g, ln_b)
    res = run_bass_kernel_spmd(nc, in_maps, core_ids=list(range(8)))
    out = np.empty((B, N, C), np.float32)
    for d in range(8):
        b, half = d // 2, d % 2
        out[b, half * NHALF:(half + 1) * NHALF, :] = res.results[d]["out"]
    return out


# revision 42
# speedup vs baseline: 1.1648x; 1.1648x over previous
"""Trainium2 Bass kernel for PVT-style spatial-reduction attention.

Reference computation (B=4, N=4096, C=512, 8 heads, head_dim=64):
  q = (x @ q_w.T) * hd**-0.5                    -> [B, N, C]
  x_ = depthwise_conv2x2_stride2(x as NCHW 64x64) + sr_b -> [B, M=1024, C]
  x_ = layernorm(x_) * ln_g + ln_b
  k, v = split(x_ @ kv_w.T)                      -> [B, nh, M, hd] each
  out = softmax(q k^T) v                         -> [B, N, C]
  out = out @ proj_w.T + proj_b
Sharding: 8 cores, core d handles batch b=d//2, query-half h=d%2 (2048
queries).  Each core computes its output slice independently (the small KV
path is recomputed per half); the host concatenates.  No collectives.

On-device layout is "transposed" (channels on partitions, tokens in the
free dimension) so every matmul contracts over the partition dim.
Attention per head-pair: S^T via row-tiled K=64 concurrent matmuls, exp on
ScalarE, PV per head with the softmax denominator folded in as a 65th
weight column of ones (head A outputs [O_A(64 rows); s_A], head B uses
[1|0*63|v_B] so O_B lands on partitions 64-127 and s_B on partition 0).
The depthwise conv runs on the PE as accumulating diagonal matmuls; the
LayerNorm affine is folded into the K/V weights host-side (softmax is
invariant to the K bias; the V bias folds into proj_b).  One exp tile per
chunk runs on the DVE via the Schraudolph bit-trick to relieve the
saturated ScalarE, with PV matmuls deferred two steps so the PE never
waits on it.  Denominator reciprocals run per chunk: DMA partition-spread
of the two s rows into [32,32], one DVE reciprocal, spread-back +
stride-0-broadcast DMAs, one deferred DVE multiply.
"""

import numpy as np

B, N, C = 4, 4096, 512
NH, HD = 8, 64
M = 1024          # (64/2) * (64/2) spatial-reduced tokens
NHALF = 2048      # queries per core
LN_EPS = 1e-5

NQT = NHALF // 128
KT = C // 128     # 4 c-tiles
MT = M // 128     # 8 m-tiles
NCH = NHALF // 512
PW = 193          # per-pair vhat width: [v_A|1] (65) + [1|0*63|v_B] (128)

_cache = {}


def _build_nc():
    import concourse.tile as tile
    from concourse import bacc, mybir
    from concourse.masks import make_identity

    f32 = mybir.dt.float32
    bf16 = mybir.dt.bfloat16
    AF = mybir.ActivationFunctionType
    OP = mybir.AluOpType

    # Pin Exp/Ln/Square to the one ACT table set that contains all three
    # (natural_log_exp_and_others); otherwise the set chooser alternates
    # between sets and pays a ~2.7us ACT_TABLE_LOAD per switch.
    import concourse.bacc as bacc_mod
    if not hasattr(bacc_mod, "_orig_get_activation_tables"):
        bacc_mod._orig_get_activation_tables = bacc_mod.get_activation_tables

        def _pinned_tables(arch):
            d = bacc_mod._orig_get_activation_tables(arch)
            strip = {AF.Exp, AF.Ln, AF.Square}
            return {
                name: (funcs if name == "natural_log_exp_and_others"
                       else funcs - strip)
                for name, funcs in d.items()
            }

        bacc_mod.get_activation_tables = _pinned_tables

    nc = bacc.Bacc("TRN2", target_bir_lowering=False, debug=False)

    x_d = nc.dram_tensor("xT", [C, N], bf16, kind="ExternalInput")
    qw_d = nc.dram_tensor("q_wT", [C, C], bf16, kind="ExternalInput")
    kw_d = nc.dram_tensor("k_wT", [C, C], bf16, kind="ExternalInput")
    vw_d = nc.dram_tensor("v_wT", [C, C], bf16, kind="ExternalInput")
    pw_d = nc.dram_tensor("p_wT", [C, C], bf16, kind="ExternalInput")
    chan_d = nc.dram_tensor("chan", [C, 8], f32, kind="ExternalInput")
    pb_d = nc.dram_tensor("p_b", [1, C], f32, kind="ExternalInput")
    out_d = nc.dram_tensor("out", [NHALF, C], f32, kind="ExternalOutput")

    with tile.TileContext(nc) as tc:
        _cms = {}

        def pool(name, bufs=1, space="SBUF", side=None):
            cm = tc.tile_pool(name=name, bufs=bufs, space=space, side=side)
            p = cm.__enter__()
            _cms[id(p)] = cm
            return p

        def close(*pools):
            for p in pools:
                _cms.pop(id(p)).__exit__(None, None, None)

        consts = pool("consts")
        ones_f = consts.tile([128, 128], f32, tag="ones_f", name="ones_f")
        nc.vector.memset(ones_f[:], 1.0)
        ones128 = consts.tile([128, 128], bf16, tag="ones128", name="ones128")
        nc.vector.tensor_copy(ones128[:], ones_f[:])
        zero_t = consts.tile([128, 1], f32, tag="zero_t", name="zero_t")
        nc.vector.memset(zero_t[:], 0.0)
        eps_t = consts.tile([128, 1], f32, tag="eps_t", name="eps_t")
        nc.vector.memset(eps_t[:], LN_EPS)
        # chan params per k-tile: cols 0-3 conv taps, 4 sr_b
        chan_t = consts.tile([128, 8 * KT], f32, tag="chan", name="chan_t")
        nc.sync.dma_start(
            chan_t[:].rearrange("p (k c) -> p k c", k=KT),
            chan_d.ap().rearrange("(k p) c -> p k c", p=128))
        chan = [chan_t[:, 8 * k:8 * (k + 1)] for k in range(KT)]
        pb_bc = consts.tile([128, C], f32, tag="pb", name="pb")
        nc.sync.dma_start(pb_bc[:], pb_d.ap().to_broadcast([128, C]))
        ident_b = consts.tile([128, 128], bf16, tag="ident", name="ident_b")
        make_identity(nc, ident_b[:])
        # per-(k, tap) diagonal conv-weight matrices: diag(chan[k][:, tap])
        diag = [[consts.tile([128, 128], bf16, tag=f"dg{k}_{t}",
                             name=f"dg{k}_{t}") for t in range(4)]
                for k in range(KT)]
        for k in range(KT):
            for t in range(4):
                nc.vector.tensor_scalar_mul(diag[k][t][:], ident_b[:],
                                            chan[k][:, t:t + 1])

        # =============== phase T: split loads -- xqT holds each k-tile's
        # query half (tokens [0,NHALF), also conv row-half 0), xTr the rest
        xT_pool = pool("xTp", side="right")
        xTr = xT_pool.tile([128, KT * NHALF], bf16, tag="xTr", name="xTr")
        xqT_pool = pool("xqTp", side="right")
        xqT = xqT_pool.tile([128, KT * NHALF], bf16, tag="xqT", name="xqT")
        pw_pool = pool("pwp")
        pw_t = pw_pool.tile([128, KT * C], bf16, tag="pw", name="pw_t")
        # kvs pool holds qT (written in phase Q) plus kTt/vhat (phase K); it
        # stays open through phase A.
        kvs = pool("kvs")
        qT = [kvs.tile([128, NHALF], bf16, tag=f"qT{k}", name=f"qT{k}")
              for k in range(KT)]
        qw_pool = pool("qwp")
        qw_t = qw_pool.tile([128, KT * C], bf16, tag="qw", name="qw_t")
        # load order: first Q-proj inputs (weights block 0 + queries), then
        # the rest; proj weights last among the prefetches
        nc.sync.dma_start(qw_t[:, 0:C], qw_d.ap()[0:128, :])
        for k in range(KT):
            nc.sync.dma_start(xqT[:, NHALF * k:NHALF * (k + 1)],
                              x_d.ap()[128 * k:128 * (k + 1), 0:NHALF])
        for k in range(1, KT):
            nc.sync.dma_start(qw_t[:, C * k:C * (k + 1)],
                              qw_d.ap()[128 * k:128 * (k + 1), :])
        for k in range(KT):
            nc.sync.dma_start(xTr[:, NHALF * k:NHALF * (k + 1)],
                              x_d.ap()[128 * k:128 * (k + 1), NHALF:N])
        nc.sync.dma_start(
            pw_t[:].rearrange("p (k j) -> p k j", k=KT),
            pw_d.ap().rearrange("(k p) j -> p k j", p=128))
        qw = [qw_t[:, C * k:C * (k + 1)] for k in range(KT)]
        pw = [pw_t[:, C * k:C * (k + 1)] for k in range(KT)]
        # =============== phase Q: q^T = q_wT.T @ xq^T ======================
        q_psum = pool("q_ps", bufs=2, space="PSUM")
        for co in range(KT):
            for ch in range(NCH):
                ps = q_psum.tile([128, 512], f32, tag="q", name="q")
                for k in range(KT):
                    nc.tensor.matmul(
                        ps[:],
                        qw[k][:, 128 * co:128 * (co + 1)],
                        xqT[:, NHALF * k + 512 * ch:
                              NHALF * k + 512 * (ch + 1)],
                        start=(k == 0), stop=(k == KT - 1),
                    )
                nc.vector.tensor_copy(qT[co][:, 512 * ch:512 * (ch + 1)],
                                      ps[:])
        close(q_psum, qw_pool)

        # =============== phase C: depthwise conv + layernorm ===============
        xn_pool = pool("xnp")
        xn = [xn_pool.tile([128, M], bf16, tag=f"xn{k}", name=f"xn{k}")
              for k in range(KT)]

        cv = pool("cv", bufs=2)
        yt_pool = pool("ytp")
        yt = [yt_pool.tile([128, M], bf16, tag=f"yt{k}", name=f"yt{k}")
              for k in range(KT)]
        st_psum = pool("st_ps", space="PSUM")
        SY = st_psum.tile([128, M], f32, tag="SY", name="SY")
        SY2 = st_psum.tile([128, M], f32, tag="SY2", name="SY2")

        cv_psum = pool("cv_ps", bufs=1, space="PSUM")
        for k in range(KT):
            # depthwise conv as 4 accumulating diagonal matmuls per half;
            # row-half 0 lives in xqT (tokens [0,NHALF)), half 1 in xTr
            acc = cv_psum.tile([128, M], f32, tag="cacc", name="cacc")
            for h, xh in ((0, xqT), (1, xTr)):
                img = xh[:, NHALF * k:NHALF * (k + 1)].rearrange(
                    "p (i a j b) -> p a b i j", i=16, a=2, j=32, b=2)
                for tap in range(4):
                    nc.tensor.matmul(
                        acc[:, 512 * h:512 * (h + 1)], diag[k][tap][:],
                        img[:, tap // 2, tap % 2],
                        start=(tap == 0), stop=(tap == 3))
            # bias add (+ cast to bf16) on ScalarE straight from PSUM
            nc.scalar.activation(yt[k][:], acc[:], AF.Identity,
                                 bias=chan[k][:, 4:5])
            ysq = cv.tile([128, M], bf16, tag="ysq", name="ysq")
            nc.vector.tensor_mul(ysq[:], yt[k][:], yt[k][:])
            for ch2 in range(M // 512):
                s_ = slice(512 * ch2, 512 * (ch2 + 1))
                nc.tensor.matmul(SY[:, s_], ones128[:], yt[k][:, s_],
                                 start=(k == 0), stop=(k == KT - 1))
                nc.tensor.matmul(SY2[:, s_], ones128[:], ysq[:, s_],
                                 start=(k == 0), stop=(k == KT - 1))
        close(cv_psum)

        # r = 1/std = exp(-0.5*ln(SY2/C - (SY/C)^2 + eps)); the division by
        # std is NOT applied to xn -- it factors out of the K/V projections
        # (contraction over c) and is applied to their outputs instead.
        mu2 = cv.tile([128, M], f32, tag="stat", name="mu2")
        nc.scalar.activation(mu2[:], SY[:], AF.Square, bias=zero_t[:],
                             scale=1.0 / C)
        var = cv.tile([128, M], f32, tag="stat", name="var")
        nc.vector.scalar_tensor_tensor(
            var[:], SY2[:], 1.0 / C, mu2[:], op0=OP.mult, op1=OP.subtract)
        lgv = cv.tile([128, M], f32, tag="stat", name="lgv")
        nc.scalar.activation(lgv[:], var[:], AF.Ln, bias=eps_t[:])
        inv_b = cv.tile([128, M], bf16, tag="invb", name="inv_b")
        nc.scalar.activation(inv_b[:], lgv[:], AF.Exp, bias=zero_t[:],
                             scale=-0.5)

        mu_neg = cv.tile([128, M], bf16, tag="mneg", name="mu_neg")
        nc.vector.tensor_scalar_mul(mu_neg[:], SY[:], -1.0 / C)
        for k in range(KT):
            t1 = cv.tile([128, M], bf16, tag="t1", name="t1")
            nc.vector.tensor_add(t1[:], mu_neg[:], yt[k][:])
            nc.vector.tensor_mul(xn[k][:], t1[:], inv_b[:])
        close(st_psum, yt_pool)
        close(xqT_pool, xT_pool)

        # =============== phase K: k^T and v-hat projections ================
        kTt = [kvs.tile([128, M], bf16, tag=f"kT{k}", name=f"kT{k}")
               for k in range(KT)]
        # vhat[mt]: [128, 4*PW] bf16, per pair p:
        #   cols p*PW    .. p*PW+63  : v_A           (head 2p)
        #   col  p*PW+64             : ones  -> s_A on out partition 64
        #   col  p*PW+65             : ones  -> s_B on out partition 0
        #   cols p*PW+66 .. p*PW+128 : zeros
        #   cols p*PW+129.. p*PW+192 : v_B           (head 2p+1)
        vhat = [kvs.tile([128, 4 * PW], bf16, tag=f"vh{m}", name=f"vh{m}")
                for m in range(MT)]
        kvw_pool = pool("kvw")
        kw_t = kvw_pool.tile([128, KT * C], bf16, tag="kw", name="kw_t")
        vw_t = kvw_pool.tile([128, KT * C], bf16, tag="vw", name="vw_t")
        nc.sync.dma_start(
            kw_t[:].rearrange("p (k j) -> p k j", k=KT),
            kw_d.ap().rearrange("(k p) j -> p k j", p=128))
        nc.sync.dma_start(
            vw_t[:].rearrange("p (k j) -> p k j", k=KT),
            vw_d.ap().rearrange("(k p) j -> p k j", p=128))
        kw = [kw_t[:, C * k:C * (k + 1)] for k in range(KT)]
        vw = [vw_t[:, C * k:C * (k + 1)] for k in range(KT)]
        kv_psum = pool("kv_ps", bufs=4, space="PSUM")
        for p in range(KT):
            for ch2 in range(M // 512):
                s_ = slice(512 * ch2, 512 * (ch2 + 1))
                kps = kv_psum.tile([128, 512], f32, tag="kv", name="kps")
                for k in range(KT):
                    nc.tensor.matmul(
                        kps[:], kw[k][:, 128 * p:128 * (p + 1)],
                        xn[k][:, s_],
                        start=(k == 0), stop=(k == KT - 1))
                nc.vector.tensor_copy(kTt[p][:, s_], kps[:])
        for mt in range(MT):
            nc.vector.memset(vhat[mt][:], 0.0)
            v4 = vhat[mt][:].rearrange("p (h x) -> p h x", h=4)
            nc.vector.memset(v4[:, :, 64:66], 1.0)
            ps = kv_psum.tile([128, 512], f32, tag="kv", name="kv")
            for k in range(KT):
                nc.tensor.matmul(
                    ps[:], xn[k][:, 128 * mt:128 * (mt + 1)], vw[k][:],
                    start=(k == 0), stop=(k == KT - 1))
            ps4 = ps[:].rearrange("p (pr two d) -> p pr two d", two=2, d=64)
            nc.vector.tensor_copy(v4[:, :, 0:64], ps4[:, :, 0, :])
            nc.vector.tensor_copy(v4[:, :, 129:193], ps4[:, :, 1, :])
        close(kv_psum)

        # =============== phase A: attention + pipelined epilogues ==========
        OT_pool = pool("OTp", side="right")
        OT = [OT_pool.tile([128, NHALF], bf16, tag=f"OT{p}", name=f"OT{p}")
              for p in range(KT)]
        sr_pool = pool("srp", side="right", bufs=2)
        ss_pool = pool("ssp", side="right", bufs=2)
        bc_pool = pool("bcp", bufs=4)

        ppool = pool("ptile", bufs=6)
        S_psum = pool("S_ps", bufs=2, space="PSUM")
        O_psum = pool("O_ps", bufs=2, space="PSUM")

        pend_pv = []  # PV states deferred two steps behind their exp

        def emit_pv(pv):
            """PV pair for one mt, deferred one step so the next QK sits
            ahead of it in the PE stream."""
            p, mt, pt, o_t = pv
            nc.tensor.matmul(
                o_t[0:65, 0:512], vhat[mt][:, PW * p:PW * p + 65],
                pt[:, 0:512],
                start=(mt == 0), stop=(mt == MT - 1))
            nc.tensor.matmul(
                o_t[0:128, 512:1024], vhat[mt][:, PW * p + 65:PW * (p + 1)],
                pt[:, 512:1024],
                start=(mt == 0), stop=(mt == MT - 1))

        def chunk_copies(st):
            """Copy O rows + s rows out of a finished chunk's PSUM.  Deferred
            until the chunk's last (deferred) PV matmul has been emitted."""
            cp, nsl, o_t, srow, _ = st
            nc.vector.tensor_copy(OT[cp][0:64, nsl], o_t[0:64, 0:512])
            nc.vector.tensor_copy(OT[cp][64:128, nsl],
                                  o_t[64:128, 512:1024])
            nc.vector.tensor_copy(srow[64:65, nsl], o_t[64:65, 0:512])
            nc.vector.tensor_copy(srow[0:1, nsl], o_t[0:1, 512:1024])

        def chunk_recip(st):
            """Per-chunk batched 1/s: gather the chunk's two s rows into
            [32,32], one DVE reciprocal, spread-back + broadcast DMAs."""
            cp, nsl, o_t, srow, st_bc = st
            sstack = ss_pool.tile([128, 32], bf16, tag="sst", name="sstack")
            nc.sync.dma_start(
                sstack[0:16, 0:32],
                srow[64:65, nsl].rearrange("p (a b) -> p a b", a=16))
            nc.sync.dma_start(
                sstack[16:32, 0:32],
                srow[0:1, nsl].rearrange("p (a b) -> p a b", a=16))
            sinv_f = ss_pool.tile([128, 32], f32, tag="sif", name="sinv_f")
            nc.vector.reciprocal(sinv_f[0:32, :], sstack[0:32, :])
            sinv_b = ss_pool.tile([128, 32], bf16, tag="sib", name="sinv_b")
            nc.vector.tensor_copy(sinv_b[0:32, :], sinv_f[0:32, :])
            sqs = ss_pool.tile([128, 512], bf16, tag="sqs", name="sqs")
            nc.sync.dma_start(
                sqs[0:1, :].rearrange("p (a b) -> p a b", a=16),
                sinv_b[0:16, :])
            nc.sync.dma_start(
                sqs[64:65, :].rearrange("p (a b) -> p a b", a=16),
                sinv_b[16:32, :])
            bc_t = bc_pool.tile([128, 512], bf16, tag="bc", name="bc")
            nc.sync.dma_start(
                bc_t[0:64, :],
                sqs[0:1, :].unsqueeze(1).broadcast_to([1, 64, 512]))
            nc.sync.dma_start(
                bc_t[64:128, :],
                sqs[64:65, :].unsqueeze(1).broadcast_to([1, 64, 512]))
            st_bc.append(bc_t)

        def chunk_mul(st):
            cp, nsl, o_t, srow, st_bc = st
            nc.vector.tensor_mul(OT[cp][:, nsl], OT[cp][:, nsl],
                                 st_bc[0][:])

        # rolling per-chunk pipeline: chunk t's PSUM->SBUF copies are
        # emitted at chunk t+1 (after its deferred last PV), the 1/s
        # pipeline right after, the normalize multiply at chunk t+2
        todo_copies = []   # chunk states awaiting copies+recip
        todo_mul = []      # chunk states awaiting the normalize multiply

        for p in range(KT):
            # srow: s_A rows live on partition 64 (col 64 of vhat-A), s_B
            # rows on partition 0; 4 chunks of 512 each.
            srow = sr_pool.tile([128, NCH * 512], bf16, tag="srow",
                                name="srow")
            for ch in range(NCH):
                nsl = slice(512 * ch, 512 * (ch + 1))
                o_t = O_psum.tile([128, 1024], f32, tag="O", name="o_t")
                for mt in range(MT):
                    msl = slice(128 * mt, 128 * (mt + 1))
                    S_ps = S_psum.tile([128, 1024], f32, tag="S", name="S_ps")
                    nc.tensor.matmul(
                        S_ps[:, 0:512],
                        kTt[p][0:64, msl], qT[p][0:64, nsl],
                        start=True, stop=True, tile_position=(0, 0))
                    nc.tensor.matmul(
                        S_ps[:, 512:1024],
                        kTt[p][64:128, msl], qT[p][64:128, nsl],
                        start=True, stop=True, tile_position=(64, 0))
                    pt = ppool.tile([128, 1024], bf16, tag="pt", name="pt")
                    if mt == 0:
                        # Schraudolph bit-trick exp on DVE for one tile per
                        # chunk (~3% sawtooth error that largely cancels
                        # between numerator and denominator); offloads the
                        # saturated ScalarE
                        tsch = ppool.tile([128, 1024], mybir.dt.int32,
                                          tag="sch", name="tsch")
                        nc.vector.tensor_scalar(
                            tsch[:], S_ps[:], 12102203.162, 1064866805.0,
                            op0=OP.mult, op1=OP.add)
                        nc.vector.tensor_copy(pt[:], tsch[:].bitcast(f32))
                    else:
                        nc.scalar.activation(pt[:], S_ps[:], AF.Exp,
                                             bias=zero_t[:])
                    if len(pend_pv) >= 2:
                        emit_pv(pend_pv.pop(0))
                    pend_pv.append((p, mt, pt, o_t))
                    if mt == 2 and todo_mul:
                        chunk_mul(todo_mul.pop(0))
                    if mt == 3 and todo_copies:
                        st = todo_copies.pop(0)
                        chunk_copies(st)
                        chunk_recip(st)
                        todo_mul.append(st)
                hist_st = (p, nsl, o_t, srow, [])
                todo_copies.append(hist_st)

        # flush the tail of the pipeline; pending multiplies go first so
        # the projection over their OT slices is not blocked behind the
        # last chunk's reciprocal chain in the DVE queue
        for pv in pend_pv:
            emit_pv(pv)
        last = todo_copies.pop(0)
        chunk_copies(last)
        for st in todo_mul:
            chunk_mul(st)
        todo_mul = []
        chunk_recip(last)
        chunk_mul(last)
        close(O_psum, S_psum, ppool, bc_pool, ss_pool, sr_pool, kvw_pool,
              cv, xn_pool, kvs)

        # =============== phase P: output projection ========================
        opool = pool("outp", bufs=3)
        pj_psum = pool("pj_ps", bufs=4, space="PSUM")
        for nt in range(NQT):
            ps = pj_psum.tile([128, 512], f32, tag="pj", name="pj")
            for p in range(KT):
                nc.tensor.matmul(
                    ps[:], OT[p][:, 128 * nt:128 * (nt + 1)], pw[p][:],
                    start=(p == 0), stop=(p == KT - 1))
            ob = opool.tile([128, 512], f32, tag="ob", name="ob")
            nc.vector.tensor_add(ob[:], ps[:], pb_bc[:])
            nc.sync.dma_start(out_d.ap()[128 * nt:128 * (nt + 1), :], ob[:])
        close(pj_psum, opool, pw_pool, OT_pool, consts)

    nc.compile()
    return nc


def _get_nc():
    if "nc" not in _cache:
        _cache["nc"] = _build_nc()
    return _cache["nc"]


def _make_in_maps(x, q_w, kv_w, proj_w, proj_b, sr_w, sr_b, ln_g, ln_b):
    import ml_dtypes
    bf = ml_dtypes.bfloat16

    x = np.asarray(x, np.float32)
    scale = HD ** -0.5
    ln_g = np.asarray(ln_g, np.float32)
    ln_b = np.asarray(ln_b, np.float32)
    proj_w = np.asarray(proj_w, np.float32)
    q_wT = np.ascontiguousarray((np.asarray(q_w, np.float32).T * scale)
                                .astype(bf))
    kv_w = np.asarray(kv_w, np.float32)
    # LayerNorm affine folded into the weights: the K-side bias drops out
    # (softmax is invariant to a per-query constant), the V-side bias is a
    # constant over tokens and folds into proj_b.
    k_wT = np.ascontiguousarray((kv_w[:C] * ln_g[None, :]).T.astype(bf))
    v_wT = np.ascontiguousarray((kv_w[C:] * ln_g[None, :]).T.astype(bf))
    p_wT = np.ascontiguousarray(proj_w.T.astype(bf))
    chan = np.zeros((C, 8), np.float32)
    sr_w = np.asarray(sr_w, np.float32)
    for di in range(2):
        for dj in range(2):
            chan[:, di * 2 + dj] = sr_w[:, 0, di, dj]
    chan[:, 4] = np.asarray(sr_b, np.float32)
    cv = kv_w[C:] @ ln_b
    p_b = (np.asarray(proj_b, np.float32) + proj_w @ cv).reshape(1, C)

    in_maps = []
    for d in range(8):
        b, half = d // 2, d % 2
        # roll so this core's query half sits at columns [0, NHALF); the
        # induced m-token permutation of the KV path is attention-invariant
        # and the roll (32 image rows) preserves the conv's 2x2 windows
        xbT = np.ascontiguousarray(
            np.roll(x[b].T, -half * NHALF, axis=1).astype(bf))
        in_maps.append({
            "xT": xbT,
            "q_wT": q_wT, "k_wT": k_wT, "v_wT": v_wT, "p_wT": p_wT,
            "chan": chan, "p_b": p_b,
        })
    return in_maps


def kernel(x, q_w, kv_w, proj_w, proj_b, sr_w, sr_b, ln_g, ln_b, H, W):
    from concourse.bass_utils import run_bass_kernel_spmd

    nc = _get_nc()
    in_maps = _make_in_maps(x, q_w, kv_w, proj_w, proj_b, sr_w, sr_b,
                            ln_g, ln_b)
    res = run_bass_kernel_spmd(nc, in_maps, core_ids=list(range(8)))
    out = np.empty((B, N, C), np.float32)
    for d in range(8):
        b, half = d // 2, d % 2
        out[b, half * NHALF:(half + 1) * NHALF, :] = res.results[d]["out"]
    return out
